# revision 36
# baseline (speedup 1.0000x reference)
"""Trainium2 Bass kernel for the batched ACT (adaptive computation time) halting
loop, data-parallel over the batch across 8 NeuronCores.

Math (per batch row b):
    s_0 = h
    s_{t+1} = [xin_t, s_t] @ W_hidden.T + b_hidden     (xin_0 = x+1, else x)
    p_t = sigmoid(s_{t+1} @ w_halt + b_halt)
    halting weights w_t from cumulative p;  sum_t w_t == 1 exactly
    hidden = sum_t w_t s_t;  output = hidden @ W_out.T + b_out  (exact rewrite)
    ponder = (n+1) + R

Rewrites:
  * xproj = x @ Wx.T + b_hidden computed once (Wx = W_hidden[:, :D_IN]);
    per-step matmul contracts only over H (K=2048 not 3072).
  * the t==0 "x+1" flag becomes adding delta = rowsum(Wx) at step 0.
  * output needs only ONE final matmul since sum_t w_t == 1.
  * halt logit from the step INPUT state: s_new.w_halt = s_t.(Ws.T w_halt)
    + (x@Wx.T + b_h).w_halt; u = Ws.T@w_halt on device (bf16), the x-part
    q (+ b_halt) in fp64 on the host -> halt path leaves the copy-dependent
    critical path.
  * state kept transposed sT [H, B_local]; transposes/reshapes host-side.
  * bf16 matmul operands (1 PE cycle/row), fp32 PSUM accumulation.
  * "big tile" layout: state/xproj/hid as single [128, 16*256] SBUF tiles.
  * k-outer matmul order for xproj and step 0 so PE chases weight DMAs.
"""

import numpy as np
import ml_dtypes

B, D_IN, H, D_OUT = 2048, 1024, 2048, 1024
T = 12
EPS = 0.01
NCORES = 8
BL = B // NCORES  # 256 rows per core
P = 128
KT_S = H // P     # 16 k-tiles over H
KT_X = D_IN // P  # 8 k-tiles over D_IN
MT = H // P       # 16 m-tiles over H
NF = 512          # moving free chunk for the final matmul

_BF16 = ml_dtypes.bfloat16


def _to_big(aT, n_free):
    """[K*128, n_free] -> [128, K*n_free], k-tile-major along free dim."""
    k = aT.shape[0] // P
    return np.ascontiguousarray(
        aT.reshape(k, P, n_free).transpose(1, 0, 2).reshape(P, k * n_free)
    )


def _from_big(big, n_free):
    """[128, K*n_free] -> [K*128, n_free]."""
    k = big.shape[1] // n_free
    return np.ascontiguousarray(
        big.reshape(P, k, n_free).transpose(1, 0, 2).reshape(k * P, n_free)
    )


def _build_nc():
    from contextlib import ExitStack

    import concourse.bass as bass
    import concourse.mybir as mybir
    import concourse.tile as tile
    from concourse import bacc

    f32 = mybir.dt.float32
    bf16 = mybir.dt.bfloat16
    ADD = mybir.AluOpType.add
    Sigmoid = mybir.ActivationFunctionType.Sigmoid

    nc = bacc.Bacc("TRN2", target_bir_lowering=False, debug=False)

    xB_d = nc.dram_tensor("xB", [P, KT_X * BL], bf16, kind="ExternalInput")
    hB_d = nc.dram_tensor("hB", [P, KT_S * BL], bf16, kind="ExternalInput")
    WxB_d = nc.dram_tensor("WxB", [P, KT_X * H], bf16, kind="ExternalInput")
    WsB_d = nc.dram_tensor("WsB", [P, KT_S * H], bf16, kind="ExternalInput")
    WoB_d = nc.dram_tensor("WoB", [P, KT_S * D_OUT], bf16, kind="ExternalInput")
    bh_d = nc.dram_tensor("bh", [P, MT], f32, kind="ExternalInput")
    delta_d = nc.dram_tensor("delta", [P, MT], f32, kind="ExternalInput")
    u_d = nc.dram_tensor("u", [P, KT_S], bf16, kind="ExternalInput")
    q_d = nc.dram_tensor("q", [1, BL], f32, kind="ExternalInput")
    q0_d = nc.dram_tensor("q0", [1, BL], f32, kind="ExternalInput")
    bout_d = nc.dram_tensor("bout", [1, D_OUT], bf16, kind="ExternalInput")

    y_d = nc.dram_tensor("y", [BL, D_OUT], f32, kind="ExternalOutput")
    hidB_d = nc.dram_tensor("hidB", [P, KT_S * BL], f32, kind="ExternalOutput")
    pond_d = nc.dram_tensor("ponder", [1, BL], f32, kind="ExternalOutput")

    def bcast_k(ap, k):
        """Broadcast a [128, n] AP to [128, k, n] via a zero-stride dim."""
        return bass.AP(
            tensor=ap.tensor, offset=ap.offset, ap=[ap.ap[0], [0, k], ap.ap[1]]
        )

    with tile.TileContext(nc) as tc, ExitStack() as ctx:
        singles = ctx.enter_context(tc.tile_pool(name="singles", bufs=1))
        wst_pool = ctx.enter_context(tc.tile_pool(name="wst", bufs=1))
        state_pool = ctx.enter_context(tc.tile_pool(name="state", bufs=3))
        xproj_pool = ctx.enter_context(tc.tile_pool(name="xproj", bufs=1))
        hid_pool = ctx.enter_context(tc.tile_pool(name="hid", bufs=1))
        tmp_pool = ctx.enter_context(tc.tile_pool(name="tmp", bufs=1))
        wbc_pool = ctx.enter_context(tc.tile_pool(name="wbc", bufs=2))
        hsmall = ctx.enter_context(tc.tile_pool(name="hsmall", bufs=8))
        cum_pool = ctx.enter_context(tc.tile_pool(name="cum", bufs=2))
        obuf_pool = ctx.enter_context(tc.tile_pool(name="obuf", bufs=4))
        psum_mm = ctx.enter_context(tc.tile_pool(name="psum_mm", bufs=6, space="PSUM"))
        psum_fin = ctx.enter_context(tc.tile_pool(name="psum_fin", bufs=2, space="PSUM"))

        # --- small constants
        bh_t = singles.tile([P, MT], f32)
        nc.sync.dma_start(bh_t[:, :], bh_d[:, :])
        delta_t = singles.tile([P, MT], f32)
        nc.sync.dma_start(delta_t[:, :], delta_d[:, :])
        u_t = singles.tile([P, KT_S], bf16)
        nc.sync.dma_start(u_t[:, :], u_d[:, :])
        q_t = singles.tile([1, BL], f32)
        nc.sync.dma_start(q_t[:, :], q_d[:, :])
        q0_t = singles.tile([1, BL], f32)
        nc.sync.dma_start(q0_t[:, :], q0_d[:, :])
        bout_t = singles.tile([1, D_OUT], bf16)
        nc.sync.dma_start(bout_t[:, :], bout_d[:, :])
        ones_t = singles.tile([1, P], bf16)
        nc.vector.memset(ones_t[:, :], 1.0)

        # warmup: keep PE active during the initial weight DMA so the HAM
        # clock gate reaches 2.4 GHz before xproj/step-0 matmuls start.
        warm_rhs = singles.tile([1, BL], bf16)
        nc.vector.memset(warm_rhs[:, :], 0.0)
        wps = psum_fin.tile([P, BL], f32, tag="fin", name="warm_ps")
        for _ in range(80):
            nc.tensor.matmul(
                wps[:, :], ones_t[:, :], warm_rhs[:, :], start=True, stop=True
            )

        # --- initial state s_0 = h (big layout, bf16)
        sbig = state_pool.tile([P, KT_S * BL], bf16, tag="state")
        nc.sync.dma_start(sbig[:, :], hB_d[:, :])

        # --- xproj = (x @ Wx.T).T + b_hidden, big layout [128, 16*256] f32
        # DMA order: xB + WxB first; k-outer matmuls chase the arriving slices.
        xpbig = xproj_pool.tile([P, KT_S * BL], f32)
        wsb = wst_pool.tile([P, KT_S * H], bf16)

        def mm_group(pairs, n_k, wtile, rtile, kw, out_cb):
            """m-outer pair-tile matmuls (steady state): psum per m-pair.

            The two halves of a pair share a PSUM bank, so their accumulation
            chains must be SEQUENTIAL — start=True clears has_written for the
            whole bank (safe only because half 0 is complete by then).
            """
            for i in pairs:
                ps = psum_mm.tile([P, 2 * BL], f32, tag="ps", name=f"ps_{i}")
                for half in (0, 1):
                    m = 2 * i + half
                    for k in range(n_k):
                        nc.tensor.matmul(
                            ps[:, half * BL : (half + 1) * BL],
                            wtile[:, k * kw + m * P : k * kw + (m + 1) * P],
                            rtile[:, k * BL : (k + 1) * BL],
                            start=(k == 0),
                            stop=(k == n_k - 1),
                        )
                out_cb(i, ps)

        with tc.tile_pool(name="xw", bufs=1) as xw_pool:
            xbt = xw_pool.tile([P, KT_X * BL], bf16, tag="xb")
            nc.sync.dma_start(xbt[:, :], xB_d[:, :])
            wxb = xw_pool.tile([P, KT_X * H], bf16, tag="wxb")
            for k in range(KT_X):
                nc.sync.dma_start(
                    wxb[:, k * H : (k + 1) * H], WxB_d[:, k * H : (k + 1) * H]
                )
            # WsT resident all steps (per-k-slice DMAs, 1MB each)
            for k in range(KT_S):
                nc.sync.dma_start(
                    wsb[:, k * H : (k + 1) * H], WsB_d[:, k * H : (k + 1) * H]
                )

            def xproj_out(i, ps):
                for half in (0, 1):
                    m = 2 * i + half
                    nc.vector.tensor_scalar_add(
                        xpbig[:, m * BL : (m + 1) * BL],
                        ps[:, half * BL : (half + 1) * BL],
                        bh_t[:, m : m + 1],
                    )

            mm_group(range(0, 8), KT_X, wxb, xbt, H, xproj_out)

        # --- W_out.T big tile (created after xw pool closes; reuses its SBUF)
        wout_pool = ctx.enter_context(tc.tile_pool(name="wout", bufs=1))
        wob = wout_pool.tile([P, KT_S * D_OUT], bf16)
        for i in range(4):
            sl = slice(i * (KT_S * D_OUT // 4), (i + 1) * (KT_S * D_OUT // 4))
            nc.sync.dma_start(wob[:, sl], WoB_d[:, sl])

        # --- halting-state accumulators
        pond = singles.tile([1, BL], f32)
        nc.vector.memset(pond[:, :], 0.0)
        cum_prev = cum_pool.tile([1, BL], f32, tag="cum")
        nc.vector.memset(cum_prev[:, :], 0.0)

        hidbig = hid_pool.tile([P, KT_S * BL], f32)
        tmpbig = tmp_pool.tile([P, KT_S * BL], f32)
        pending_hid_add = [False]

        def flush_hid_add():
            # deferred hid += tmp from the PREVIOUS step: emitted after this
            # step's psum evacuations so the (gpsimd-mult-gated) DVE add can't
            # head-of-line block them.
            if pending_hid_add[0]:
                nc.vector.tensor_add(hidbig[:, :], hidbig[:, :], tmpbig[:, :])
                pending_hid_add[0] = False

        # --- the 12-step recurrence
        for t in range(T):
            # halt logit from the INPUT state: p_t = sigmoid(s_t.u + q).
            # two col-groups in separate PSUM banks run concurrently
            # (tile_position col strips), halving the M=1 matmul serial span.
            hpsA = psum_fin.tile([P, BL], f32, tag="fin", name=f"hpsA_{t}")
            hpsB = psum_fin.tile([P, BL], f32, tag="fin", name=f"hpsB_{t}")
            for r in range(KT_S // 2):
                nc.tensor.matmul(
                    hpsA[0:1, :],
                    u_t[:, r : r + 1],
                    sbig[:, r * BL : (r + 1) * BL],
                    start=(r == 0),
                    stop=(r == KT_S // 2 - 1),
                    tile_position=(0, 0),
                )
                k = KT_S // 2 + r
                nc.tensor.matmul(
                    hpsB[32:33, :],
                    u_t[:, k : k + 1],
                    sbig[:, k * BL : (k + 1) * BL],
                    start=(r == 0),
                    stop=(r == KT_S // 2 - 1),
                    tile_position=(0, 32),
                )
            ha = hsmall.tile([1, BL], f32, tag="hs")
            nc.vector.tensor_add(
                ha[:, :], (q0_t if t == 0 else q_t)[:, :], hpsA[0:1, :]
            )
            lg = hsmall.tile([1, BL], f32, tag="hs")
            nc.vector.tensor_add(lg[:, :], ha[:, :], hpsB[32:33, :])
            p_t = hsmall.tile([1, BL], f32, tag="hs")
            nc.scalar.activation(p_t[:, :], lg[:, :], Sigmoid)

            # online halting weights (all [1, BL], partition 0)
            notdone = hsmall.tile([1, BL], f32, tag="hs")
            nc.vector.tensor_scalar(
                notdone[:, :], cum_prev[:, :], 1.0 - EPS, None,
                op0=mybir.AluOpType.is_lt,
            )
            rterm = hsmall.tile([1, BL], f32, tag="hs")
            nc.vector.tensor_scalar(
                rterm[:, :], cum_prev[:, :], -1.0, 1.0,
                op0=mybir.AluOpType.mult, op1=ADD,
            )
            cum_new = cum_pool.tile([1, BL], f32, tag="cum")
            nc.vector.tensor_add(cum_new[:, :], cum_prev[:, :], p_t[:, :])
            col = hsmall.tile([1, BL], f32, tag="hs")
            if t < T - 1:
                nc.vector.tensor_scalar(
                    col[:, :], cum_new[:, :], 1.0 - EPS, None,
                    op0=mybir.AluOpType.is_ge,
                )
            else:
                nc.vector.memset(col[:, :], 1.0)
            # sel = p + col*(rterm - p);  w = sel * notdone
            suba = hsmall.tile([1, BL], f32, tag="hs")
            nc.vector.tensor_sub(suba[:, :], rterm[:, :], p_t[:, :])
            selt = hsmall.tile([1, BL], f32, tag="hs")
            nc.vector.tensor_mul(selt[:, :], col[:, :], suba[:, :])
            sel = hsmall.tile([1, BL], f32, tag="hs")
            nc.vector.tensor_add(sel[:, :], selt[:, :], p_t[:, :])
            w_t = hsmall.tile([1, BL], f32, tag="hs")
            nc.vector.tensor_mul(w_t[:, :], sel[:, :], notdone[:, :])
            isn = hsmall.tile([1, BL], f32, tag="hs")
            nc.vector.tensor_mul(isn[:, :], col[:, :], notdone[:, :])
            nc.vector.tensor_add(pond[:, :], pond[:, :], notdone[:, :])
            tmp2 = hsmall.tile([1, BL], f32, tag="hs")
            nc.vector.tensor_mul(tmp2[:, :], isn[:, :], rterm[:, :])
            nc.vector.tensor_add(pond[:, :], pond[:, :], tmp2[:, :])
            wbc = wbc_pool.tile([P, BL], f32, tag="wbc")
            nc.gpsimd.partition_broadcast(wbc[:, :], w_t[:, :])

            # hidden-state matmuls s_new = Ws @ s_t (+ xproj [+ delta at t=0])
            snew = state_pool.tile([P, KT_S * BL], bf16, tag="state")

            def step_out(i, ps, t=t, snew=snew):
                if t == 0:
                    for half in (0, 1):
                        m = 2 * i + half
                        nc.vector.scalar_tensor_tensor(
                            snew[:, m * BL : (m + 1) * BL],
                            ps[:, half * BL : (half + 1) * BL],
                            delta_t[:, m : m + 1],
                            xpbig[:, m * BL : (m + 1) * BL],
                            op0=ADD,
                            op1=ADD,
                        )
                else:
                    nc.vector.tensor_add(
                        snew[:, i * 2 * BL : (i + 1) * 2 * BL],
                        ps[:, :],
                        xpbig[:, i * 2 * BL : (i + 1) * 2 * BL],
                    )

            mm_group(range(0, 8), KT_S, wsb, sbig, H, step_out)
            flush_hid_add()

            # accumulate hidden: hid += w_t * s_new
            s3 = snew[:, :].rearrange("p (k b) -> p k b", k=KT_S)
            wb3 = bcast_k(wbc[:, :], KT_S)
            if t == 0:
                h3 = hidbig[:, :].rearrange("p (k b) -> p k b", k=KT_S)
                nc.gpsimd.tensor_mul(h3, s3, wb3)
            elif t < T - 1:
                t3 = tmpbig[:, :].rearrange("p (k b) -> p k b", k=KT_S)
                nc.gpsimd.tensor_mul(t3, s3, wb3)
                pending_hid_add[0] = True
            else:
                # last step: quarter-split (mult on GPSIMD, add on DVE) so the
                # final matmul's k-groups start as soon as quarters complete.
                hid_bf = state_pool.tile([P, KT_S * BL], bf16, tag="state")
                Q = 4
                KQ = KT_S // Q
                QW = KQ * BL
                for q in range(Q):
                    sl = slice(q * QW, (q + 1) * QW)
                    sq = snew[:, sl].rearrange("p (k b) -> p k b", k=KQ)
                    tq = tmpbig[:, sl].rearrange("p (k b) -> p k b", k=KQ)
                    eng = nc.vector if q % 2 == 0 else nc.gpsimd
                    eng.tensor_mul(tq, sq, bcast_k(wbc[:, :], KQ))
                    nc.vector.tensor_add(
                        hid_bf[:, sl], hidbig[:, sl], tmpbig[:, sl]
                    )
                # f32 hidden for the hidB output (off the y critical path)
                nc.vector.tensor_add(hidbig[:, :], hidbig[:, :], tmpbig[:, :])

            cum_prev = cum_new
            sbig = snew

        # --- outputs: hidden (big f32), ponder, output = hidden @ W_out.T + b_out
        nc.sync.dma_start(pond_d[:, :], pond[:, :])
        nc.sync.dma_start(hidB_d[:, :], hidbig[:, :])

        # final matmul: 4 psum groups, contraction emitted quarter-phase so PE
        # starts each 4-k chunk as soon as that hid_bf quarter is ready.
        groups = [(mb, nk) for mb in range(BL // P) for nk in range(D_OUT // NF)]
        psos = [
            psum_mm.tile([P, NF], f32, tag="ps", name=f"pso_{g}")
            for g in range(len(groups))
        ]
        for q in range(4):
            for g, (mb, nk) in enumerate(groups):
                for k in range(q * (KT_S // 4), (q + 1) * (KT_S // 4)):
                    nc.tensor.matmul(
                        psos[g][:, :],
                        hid_bf[:, k * BL + mb * P : k * BL + (mb + 1) * P],
                        wob[:, k * D_OUT + nk * NF : k * D_OUT + (nk + 1) * NF],
                        start=(k == 0),
                        stop=False,
                    )
        for g, (mb, nk) in enumerate(groups):
            # += b_out via a K=1 ones-row matmul (keeps the epilogue off DVE)
            nc.tensor.matmul(
                psos[g][:, :],
                ones_t[:, :],
                bout_t[:, nk * NF : (nk + 1) * NF],
                start=False,
                stop=True,
            )
            ob = obuf_pool.tile([P, NF], f32, tag="ob", name=f"ob_{g}")
            nc.vector.tensor_copy(ob[:, :], psos[g][:, :])
            nc.sync.dma_start(
                y_d[mb * P : (mb + 1) * P, nk * NF : (nk + 1) * NF], ob[:, :]
            )

    nc.compile()
    return nc


def _prep_inputs(x, h, W_hidden, b_hidden, w_halt, b_halt, W_out, b_out):
    f32, f64 = np.float32, np.float64
    WT = np.ascontiguousarray(W_hidden.T).astype(_BF16)  # [3072, 2048]
    WxB = _to_big(WT[:D_IN], H)
    WsB = _to_big(WT[D_IN:], H)
    WoB = _to_big(np.ascontiguousarray(W_out.T).astype(_BF16), D_OUT)
    Wx = W_hidden[:, :D_IN]
    delta = Wx.astype(f32).sum(axis=1, dtype=f32)
    bh_t = np.ascontiguousarray(b_hidden.astype(f32).reshape(MT, P).T)
    delta_t = np.ascontiguousarray(delta.reshape(MT, P).T)
    # halt-path host vectors (fp64): u = Ws.T @ w_halt, q = x@(Wx.T@w_halt) + c
    wh64 = w_halt.astype(f64)
    u = (W_hidden[:, D_IN:].T.astype(f64) @ wh64).astype(f32)
    v = Wx.T.astype(f64) @ wh64
    cq = float(b_hidden.astype(f64) @ wh64) + float(b_halt)
    dq = float(delta.astype(f64) @ wh64)
    u_t = np.ascontiguousarray(u.reshape(KT_S, P).T).astype(_BF16)
    bout_b = b_out.astype(_BF16).reshape(1, D_OUT)
    in_maps = []
    for c in range(NCORES):
        xs = x[c * BL : (c + 1) * BL]
        hs = h[c * BL : (c + 1) * BL]
        q = (xs.astype(f64) @ v + cq).astype(f32).reshape(1, BL)
        in_maps.append(
            {
                "xB": _to_big(np.ascontiguousarray(xs.T).astype(_BF16), BL),
                "hB": _to_big(np.ascontiguousarray(hs.T).astype(_BF16), BL),
                "WxB": WxB,
                "WsB": WsB,
                "WoB": WoB,
                "bh": bh_t,
                "delta": delta_t,
                "u": u_t,
                "q": q,
                "q0": (q + np.float32(dq)).astype(f32),
                "bout": bout_b,
            }
        )
    return in_maps


_NC_CACHE = {}


def _get_nc():
    if "nc" not in _NC_CACHE:
        _NC_CACHE["nc"] = _build_nc()
    return _NC_CACHE["nc"]


def _gather(outs):
    y = np.concatenate([np.asarray(outs[c]["y"]) for c in range(NCORES)], axis=0)
    hidden = np.concatenate(
        [_from_big(np.asarray(outs[c]["hidB"]), BL).T for c in range(NCORES)], axis=0
    )
    ponder = np.concatenate(
        [np.asarray(outs[c]["ponder"]).reshape(-1) for c in range(NCORES)], axis=0
    )
    return (
        y.astype(np.float32),
        np.ascontiguousarray(hidden, dtype=np.float32),
        ponder.astype(np.float32),
    )


def kernel(x, h, W_hidden, b_hidden, w_halt, b_halt, W_out, b_out):
    from concourse.bass_utils import run_bass_kernel_spmd

    in_maps = _prep_inputs(
        np.asarray(x), np.asarray(h), np.asarray(W_hidden), np.asarray(b_hidden),
        np.asarray(w_halt), np.asarray(b_halt), np.asarray(W_out), np.asarray(b_out),
    )
    nc = _get_nc()
    res = run_bass_kernel_spmd(nc, in_maps, core_ids=list(range(NCORES)))
    return _gather(res.results)


# revision 37
# speedup vs baseline: 1.2112x; 1.2112x over previous
"""Trainium2 Bass kernel for the batched ACT (adaptive computation time) halting
loop, data-parallel over the batch across 8 NeuronCores.

Math (per batch row b):
    s_0 = h
    s_{t+1} = [xin_t, s_t] @ W_hidden.T + b_hidden     (xin_0 = x+1, else x)
    p_t = sigmoid(s_{t+1} @ w_halt + b_halt)
    halting weights w_t from cumulative p;  sum_t w_t == 1 exactly
    hidden = sum_t w_t s_t;  output = hidden @ W_out.T + b_out  (exact rewrite)
    ponder = (n+1) + R

Rewrites:
  * xproj = x @ Wx.T + b_hidden computed once (Wx = W_hidden[:, :D_IN]);
    per-step matmul contracts only over H (K=2048 not 3072).
  * the t==0 "x+1" flag becomes adding delta = rowsum(Wx) at step 0.
  * output needs only ONE final matmul since sum_t w_t == 1.
  * halt logit from the step INPUT state: s_new.w_halt = s_t.(Ws.T w_halt)
    + (x@Wx.T + b_h).w_halt; u = Ws.T@w_halt on device (bf16), the x-part
    q (+ b_halt) in fp64 on the host -> halt path leaves the copy-dependent
    critical path.
  * state kept transposed sT [H, B_local]; transposes/reshapes host-side.
  * bf16 matmul operands (1 PE cycle/row), fp32 PSUM accumulation.
  * "big tile" layout: state/xproj/hid as single [128, 16*256] SBUF tiles.
  * k-outer matmul order for xproj and step 0 so PE chases weight DMAs.
"""

import numpy as np
import ml_dtypes

B, D_IN, H, D_OUT = 2048, 1024, 2048, 1024
T = 12
EPS = 0.01
NCORES = 8
BL = B // NCORES  # 256 rows per core
P = 128
KT_S = H // P     # 16 k-tiles over H
KT_X = D_IN // P  # 8 k-tiles over D_IN
MT = H // P       # 16 m-tiles over H
NF = 512          # moving free chunk for the final matmul

_BF16 = ml_dtypes.bfloat16


def _to_big(aT, n_free):
    """[K*128, n_free] -> [128, K*n_free], k-tile-major along free dim."""
    k = aT.shape[0] // P
    return np.ascontiguousarray(
        aT.reshape(k, P, n_free).transpose(1, 0, 2).reshape(P, k * n_free)
    )


def _from_big(big, n_free):
    """[128, K*n_free] -> [K*128, n_free]."""
    k = big.shape[1] // n_free
    return np.ascontiguousarray(
        big.reshape(P, k, n_free).transpose(1, 0, 2).reshape(k * P, n_free)
    )


def _build_nc():
    from contextlib import ExitStack

    import concourse.bass as bass
    import concourse.mybir as mybir
    import concourse.tile as tile
    from concourse import bacc

    f32 = mybir.dt.float32
    bf16 = mybir.dt.bfloat16
    ADD = mybir.AluOpType.add
    Sigmoid = mybir.ActivationFunctionType.Sigmoid

    nc = bacc.Bacc("TRN2", target_bir_lowering=False, debug=False)

    xB_d = nc.dram_tensor("xB", [P, KT_X * BL], bf16, kind="ExternalInput")
    hB_d = nc.dram_tensor("hB", [P, KT_S * BL], bf16, kind="ExternalInput")
    WxB_d = nc.dram_tensor("WxB", [P, KT_X * H], bf16, kind="ExternalInput")
    WsB_d = nc.dram_tensor("WsB", [P, KT_S * H], bf16, kind="ExternalInput")
    WoB_d = nc.dram_tensor("WoB", [P, KT_S * D_OUT], bf16, kind="ExternalInput")
    bh_d = nc.dram_tensor("bh", [P, MT], f32, kind="ExternalInput")
    delta_d = nc.dram_tensor("delta", [P, MT], f32, kind="ExternalInput")
    u_d = nc.dram_tensor("u", [P, KT_S], bf16, kind="ExternalInput")
    q_d = nc.dram_tensor("q", [1, BL], f32, kind="ExternalInput")
    q0_d = nc.dram_tensor("q0", [1, BL], f32, kind="ExternalInput")
    bout_d = nc.dram_tensor("bout", [1, D_OUT], bf16, kind="ExternalInput")

    y_d = nc.dram_tensor("y", [BL, D_OUT], f32, kind="ExternalOutput")
    hidB_d = nc.dram_tensor("hidB", [P, KT_S * BL], f32, kind="ExternalOutput")
    pond_d = nc.dram_tensor("ponder", [1, BL], f32, kind="ExternalOutput")

    def bcast_k(ap, k):
        """Broadcast a [128, n] AP to [128, k, n] via a zero-stride dim."""
        return bass.AP(
            tensor=ap.tensor, offset=ap.offset, ap=[ap.ap[0], [0, k], ap.ap[1]]
        )

    with tile.TileContext(nc) as tc, ExitStack() as ctx:
        singles = ctx.enter_context(tc.tile_pool(name="singles", bufs=1))
        wst_pool = ctx.enter_context(tc.tile_pool(name="wst", bufs=1))
        state_pool = ctx.enter_context(tc.tile_pool(name="state", bufs=3))
        xproj_pool = ctx.enter_context(tc.tile_pool(name="xproj", bufs=1))
        hid_pool = ctx.enter_context(tc.tile_pool(name="hid", bufs=1))
        tmp_pool = ctx.enter_context(tc.tile_pool(name="tmp", bufs=1))
        wbc_pool = ctx.enter_context(tc.tile_pool(name="wbc", bufs=2))
        hsmall = ctx.enter_context(tc.tile_pool(name="hsmall", bufs=8))
        cum_pool = ctx.enter_context(tc.tile_pool(name="cum", bufs=2))
        obuf_pool = ctx.enter_context(tc.tile_pool(name="obuf", bufs=4))
        psum_mm = ctx.enter_context(tc.tile_pool(name="psum_mm", bufs=6, space="PSUM"))
        psum_fin = ctx.enter_context(tc.tile_pool(name="psum_fin", bufs=2, space="PSUM"))

        # --- small constants
        bh_t = singles.tile([P, MT], f32)
        nc.sync.dma_start(bh_t[:, :], bh_d[:, :])
        delta_t = singles.tile([P, MT], f32)
        nc.sync.dma_start(delta_t[:, :], delta_d[:, :])
        u_t = singles.tile([P, KT_S], bf16)
        nc.sync.dma_start(u_t[:, :], u_d[:, :])
        q_t = singles.tile([1, BL], f32)
        nc.sync.dma_start(q_t[:, :], q_d[:, :])
        q0_t = singles.tile([1, BL], f32)
        nc.sync.dma_start(q0_t[:, :], q0_d[:, :])
        bout_t = singles.tile([1, D_OUT], bf16)
        nc.sync.dma_start(bout_t[:, :], bout_d[:, :])
        ones_t = singles.tile([1, P], bf16)
        nc.vector.memset(ones_t[:, :], 1.0)

        # warmup: keep PE active during the initial weight DMA so the HAM
        # clock gate reaches 2.4 GHz before xproj/step-0 matmuls start.
        warm_rhs = singles.tile([1, BL], bf16)
        nc.vector.memset(warm_rhs[:, :], 0.0)
        wps = psum_fin.tile([P, BL], f32, tag="fin", name="warm_ps")
        for _ in range(80):
            nc.tensor.matmul(
                wps[:, :], ones_t[:, :], warm_rhs[:, :], start=True, stop=True
            )

        # --- initial state s_0 = h (big layout, bf16)
        sbig = state_pool.tile([P, KT_S * BL], bf16, tag="state")
        nc.sync.dma_start(sbig[:, :], hB_d[:, :])

        # --- xproj = (x @ Wx.T).T + b_hidden, big layout [128, 16*256] f32
        # DMA order: xB + WxB first; k-outer matmuls chase the arriving slices.
        xpbig = xproj_pool.tile([P, KT_S * BL], f32)
        wsb = wst_pool.tile([P, KT_S * H], bf16)

        def mm_group(pairs, n_k, wtile, rtile, kw, out_cb):
            """m-outer pair-tile matmuls (steady state): psum per m-pair.

            The two halves of a pair share a PSUM bank, so their accumulation
            chains must be SEQUENTIAL — start=True clears has_written for the
            whole bank (safe only because half 0 is complete by then).
            """
            for i in pairs:
                ps = psum_mm.tile([P, 2 * BL], f32, tag="ps", name=f"ps_{i}")
                for half in (0, 1):
                    m = 2 * i + half
                    for k in range(n_k):
                        nc.tensor.matmul(
                            ps[:, half * BL : (half + 1) * BL],
                            wtile[:, k * kw + m * P : k * kw + (m + 1) * P],
                            rtile[:, k * BL : (k + 1) * BL],
                            start=(k == 0),
                            stop=(k == n_k - 1),
                        )
                out_cb(i, ps)

        with tc.tile_pool(name="xw", bufs=1) as xw_pool:
            xbt = xw_pool.tile([P, KT_X * BL], bf16, tag="xb")
            nc.sync.dma_start(xbt[:, :], xB_d[:, :])
            wxb = xw_pool.tile([P, KT_X * H], bf16, tag="wxb")
            for k in range(KT_X):
                nc.sync.dma_start(
                    wxb[:, k * H : (k + 1) * H], WxB_d[:, k * H : (k + 1) * H]
                )
            # WsT resident all steps (per-k-slice DMAs, 1MB each)
            for k in range(KT_S):
                nc.sync.dma_start(
                    wsb[:, k * H : (k + 1) * H], WsB_d[:, k * H : (k + 1) * H]
                )

            def xproj_out(i, ps):
                for half in (0, 1):
                    m = 2 * i + half
                    nc.vector.tensor_scalar_add(
                        xpbig[:, m * BL : (m + 1) * BL],
                        ps[:, half * BL : (half + 1) * BL],
                        bh_t[:, m : m + 1],
                    )

            mm_group(range(0, 8), KT_X, wxb, xbt, H, xproj_out)

        # --- W_out.T big tile (created after xw pool closes; reuses its SBUF)
        wout_pool = ctx.enter_context(tc.tile_pool(name="wout", bufs=1))
        wob = wout_pool.tile([P, KT_S * D_OUT], bf16)
        for i in range(4):
            sl = slice(i * (KT_S * D_OUT // 4), (i + 1) * (KT_S * D_OUT // 4))
            nc.sync.dma_start(wob[:, sl], WoB_d[:, sl])

        # --- halting-state accumulators
        pond = singles.tile([1, BL], f32)
        nc.vector.memset(pond[:, :], 0.0)
        cum_prev = cum_pool.tile([1, BL], f32, tag="cum")
        nc.vector.memset(cum_prev[:, :], 0.0)

        hidbig = hid_pool.tile([P, KT_S * BL], f32)
        tmpbig = tmp_pool.tile([P, KT_S * BL], f32)
        pending_hid_add = [False]

        def flush_hid_add():
            # deferred hid += tmp from the PREVIOUS step: emitted after this
            # step's psum evacuations so the (gpsimd-mult-gated) DVE add can't
            # head-of-line block them.
            if pending_hid_add[0]:
                nc.vector.tensor_add(hidbig[:, :], hidbig[:, :], tmpbig[:, :])
                pending_hid_add[0] = False

        # --- the 12-step recurrence
        for t in range(T):
            # halt logit from the INPUT state: p_t = sigmoid(s_t.u + q).
            # two col-groups in separate PSUM banks run concurrently
            # (tile_position col strips), halving the M=1 matmul serial span.
            hpsA = psum_fin.tile([P, BL], f32, tag="fin", name=f"hpsA_{t}")
            hpsB = psum_fin.tile([P, BL], f32, tag="fin", name=f"hpsB_{t}")
            for r in range(KT_S // 2):
                nc.tensor.matmul(
                    hpsA[0:1, :],
                    u_t[:, r : r + 1],
                    sbig[:, r * BL : (r + 1) * BL],
                    start=(r == 0),
                    stop=(r == KT_S // 2 - 1),
                    tile_position=(0, 0),
                )
                k = KT_S // 2 + r
                nc.tensor.matmul(
                    hpsB[32:33, :],
                    u_t[:, k : k + 1],
                    sbig[:, k * BL : (k + 1) * BL],
                    start=(r == 0),
                    stop=(r == KT_S // 2 - 1),
                    tile_position=(0, 32),
                )
            ha = hsmall.tile([1, BL], f32, tag="hs")
            nc.vector.tensor_add(
                ha[:, :], (q0_t if t == 0 else q_t)[:, :], hpsA[0:1, :]
            )
            lg = hsmall.tile([1, BL], f32, tag="hs")
            nc.vector.tensor_add(lg[:, :], ha[:, :], hpsB[32:33, :])
            p_t = hsmall.tile([1, BL], f32, tag="hs")
            nc.scalar.activation(p_t[:, :], lg[:, :], Sigmoid)

            # online halting weights (all [1, BL], partition 0)
            notdone = hsmall.tile([1, BL], f32, tag="hs")
            nc.vector.tensor_scalar(
                notdone[:, :], cum_prev[:, :], 1.0 - EPS, None,
                op0=mybir.AluOpType.is_lt,
            )
            rterm = hsmall.tile([1, BL], f32, tag="hs")
            nc.vector.tensor_scalar(
                rterm[:, :], cum_prev[:, :], -1.0, 1.0,
                op0=mybir.AluOpType.mult, op1=ADD,
            )
            cum_new = cum_pool.tile([1, BL], f32, tag="cum")
            nc.vector.tensor_add(cum_new[:, :], cum_prev[:, :], p_t[:, :])
            col = hsmall.tile([1, BL], f32, tag="hs")
            if t < T - 1:
                nc.vector.tensor_scalar(
                    col[:, :], cum_new[:, :], 1.0 - EPS, None,
                    op0=mybir.AluOpType.is_ge,
                )
            else:
                nc.vector.memset(col[:, :], 1.0)
            # sel = p + col*(rterm - p);  w = sel * notdone
            suba = hsmall.tile([1, BL], f32, tag="hs")
            nc.vector.tensor_sub(suba[:, :], rterm[:, :], p_t[:, :])
            selt = hsmall.tile([1, BL], f32, tag="hs")
            nc.vector.tensor_mul(selt[:, :], col[:, :], suba[:, :])
            sel = hsmall.tile([1, BL], f32, tag="hs")
            nc.vector.tensor_add(sel[:, :], selt[:, :], p_t[:, :])
            w_t = hsmall.tile([1, BL], f32, tag="hs")
            nc.vector.tensor_mul(w_t[:, :], sel[:, :], notdone[:, :])
            isn = hsmall.tile([1, BL], f32, tag="hs")
            nc.vector.tensor_mul(isn[:, :], col[:, :], notdone[:, :])
            nc.vector.tensor_add(pond[:, :], pond[:, :], notdone[:, :])
            tmp2 = hsmall.tile([1, BL], f32, tag="hs")
            nc.vector.tensor_mul(tmp2[:, :], isn[:, :], rterm[:, :])
            nc.vector.tensor_add(pond[:, :], pond[:, :], tmp2[:, :])
            wbc = wbc_pool.tile([P, BL], f32, tag="wbc")
            nc.gpsimd.partition_broadcast(wbc[:, :], w_t[:, :])

            # hidden-state matmuls s_new = Ws @ s_t (+ xproj [+ delta at t=0])
            snew = state_pool.tile([P, KT_S * BL], bf16, tag="state")

            def step_out(i, ps, t=t, snew=snew):
                if t == 0:
                    for half in (0, 1):
                        m = 2 * i + half
                        nc.vector.scalar_tensor_tensor(
                            snew[:, m * BL : (m + 1) * BL],
                            ps[:, half * BL : (half + 1) * BL],
                            delta_t[:, m : m + 1],
                            xpbig[:, m * BL : (m + 1) * BL],
                            op0=ADD,
                            op1=ADD,
                        )
                else:
                    nc.vector.tensor_add(
                        snew[:, i * 2 * BL : (i + 1) * 2 * BL],
                        ps[:, :],
                        xpbig[:, i * 2 * BL : (i + 1) * 2 * BL],
                    )

            mm_group(range(0, 8), KT_S, wsb, sbig, H, step_out)
            flush_hid_add()

            # accumulate hidden: hid += w_t * s_new
            s3 = snew[:, :].rearrange("p (k b) -> p k b", k=KT_S)
            wb3 = bcast_k(wbc[:, :], KT_S)
            if t == 0:
                h3 = hidbig[:, :].rearrange("p (k b) -> p k b", k=KT_S)
                nc.gpsimd.tensor_mul(h3, s3, wb3)
            elif t < T - 1:
                t3 = tmpbig[:, :].rearrange("p (k b) -> p k b", k=KT_S)
                nc.gpsimd.tensor_mul(t3, s3, wb3)
                pending_hid_add[0] = True
            else:
                # last step: quarter-split (mult on GPSIMD, add on DVE) so the
                # final matmul's k-groups start as soon as quarters complete.
                hid_bf = state_pool.tile([P, KT_S * BL], bf16, tag="state")
                Q = 4
                KQ = KT_S // Q
                QW = KQ * BL
                # all quarter ops on DVE: a gpsimd-gated op in the in-order
                # DVE queue can get scheduled ahead of this step's psum
                # copies and stall PE (measured 5-8us priority inversion).
                for q in range(Q):
                    sl = slice(q * QW, (q + 1) * QW)
                    sq = snew[:, sl].rearrange("p (k b) -> p k b", k=KQ)
                    tq = tmpbig[:, sl].rearrange("p (k b) -> p k b", k=KQ)
                    nc.vector.tensor_mul(tq, sq, bcast_k(wbc[:, :], KQ))
                    nc.vector.tensor_add(
                        hid_bf[:, sl], hidbig[:, sl], tmpbig[:, sl]
                    )
                # f32 hidden for the hidB output (off the y critical path)
                nc.vector.tensor_add(hidbig[:, :], hidbig[:, :], tmpbig[:, :])

            cum_prev = cum_new
            sbig = snew

        # --- outputs: hidden (big f32), ponder, output = hidden @ W_out.T + b_out
        nc.sync.dma_start(pond_d[:, :], pond[:, :])
        nc.sync.dma_start(hidB_d[:, :], hidbig[:, :])

        # final matmul: 4 psum groups, contraction emitted quarter-phase so PE
        # starts each 4-k chunk as soon as that hid_bf quarter is ready.
        groups = [(mb, nk) for mb in range(BL // P) for nk in range(D_OUT // NF)]
        psos = [
            psum_mm.tile([P, NF], f32, tag="ps", name=f"pso_{g}")
            for g in range(len(groups))
        ]
        for q in range(4):
            for g, (mb, nk) in enumerate(groups):
                for k in range(q * (KT_S // 4), (q + 1) * (KT_S // 4)):
                    nc.tensor.matmul(
                        psos[g][:, :],
                        hid_bf[:, k * BL + mb * P : k * BL + (mb + 1) * P],
                        wob[:, k * D_OUT + nk * NF : k * D_OUT + (nk + 1) * NF],
                        start=(k == 0),
                        stop=False,
                    )
        for g, (mb, nk) in enumerate(groups):
            # += b_out via a K=1 ones-row matmul (keeps the epilogue off DVE)
            nc.tensor.matmul(
                psos[g][:, :],
                ones_t[:, :],
                bout_t[:, nk * NF : (nk + 1) * NF],
                start=False,
                stop=True,
            )
            ob = obuf_pool.tile([P, NF], f32, tag="ob", name=f"ob_{g}")
            nc.vector.tensor_copy(ob[:, :], psos[g][:, :])
            nc.sync.dma_start(
                y_d[mb * P : (mb + 1) * P, nk * NF : (nk + 1) * NF], ob[:, :]
            )

    nc.compile()
    return nc


def _prep_inputs(x, h, W_hidden, b_hidden, w_halt, b_halt, W_out, b_out):
    f32, f64 = np.float32, np.float64
    WT = np.ascontiguousarray(W_hidden.T).astype(_BF16)  # [3072, 2048]
    WxB = _to_big(WT[:D_IN], H)
    WsB = _to_big(WT[D_IN:], H)
    WoB = _to_big(np.ascontiguousarray(W_out.T).astype(_BF16), D_OUT)
    Wx = W_hidden[:, :D_IN]
    delta = Wx.astype(f32).sum(axis=1, dtype=f32)
    bh_t = np.ascontiguousarray(b_hidden.astype(f32).reshape(MT, P).T)
    delta_t = np.ascontiguousarray(delta.reshape(MT, P).T)
    # halt-path host vectors (fp64): u = Ws.T @ w_halt, q = x@(Wx.T@w_halt) + c
    wh64 = w_halt.astype(f64)
    u = (W_hidden[:, D_IN:].T.astype(f64) @ wh64).astype(f32)
    v = Wx.T.astype(f64) @ wh64
    cq = float(b_hidden.astype(f64) @ wh64) + float(b_halt)
    dq = float(delta.astype(f64) @ wh64)
    u_t = np.ascontiguousarray(u.reshape(KT_S, P).T).astype(_BF16)
    bout_b = b_out.astype(_BF16).reshape(1, D_OUT)
    in_maps = []
    for c in range(NCORES):
        xs = x[c * BL : (c + 1) * BL]
        hs = h[c * BL : (c + 1) * BL]
        q = (xs.astype(f64) @ v + cq).astype(f32).reshape(1, BL)
        in_maps.append(
            {
                "xB": _to_big(np.ascontiguousarray(xs.T).astype(_BF16), BL),
                "hB": _to_big(np.ascontiguousarray(hs.T).astype(_BF16), BL),
                "WxB": WxB,
                "WsB": WsB,
                "WoB": WoB,
                "bh": bh_t,
                "delta": delta_t,
                "u": u_t,
                "q": q,
                "q0": (q + np.float32(dq)).astype(f32),
                "bout": bout_b,
            }
        )
    return in_maps


_NC_CACHE = {}


def _get_nc():
    if "nc" not in _NC_CACHE:
        _NC_CACHE["nc"] = _build_nc()
    return _NC_CACHE["nc"]


def _gather(outs):
    y = np.concatenate([np.asarray(outs[c]["y"]) for c in range(NCORES)], axis=0)
    hidden = np.concatenate(
        [_from_big(np.asarray(outs[c]["hidB"]), BL).T for c in range(NCORES)], axis=0
    )
    ponder = np.concatenate(
        [np.asarray(outs[c]["ponder"]).reshape(-1) for c in range(NCORES)], axis=0
    )
    return (
        y.astype(np.float32),
        np.ascontiguousarray(hidden, dtype=np.float32),
        ponder.astype(np.float32),
    )


def kernel(x, h, W_hidden, b_hidden, w_halt, b_halt, W_out, b_out):
    from concourse.bass_utils import run_bass_kernel_spmd

    in_maps = _prep_inputs(
        np.asarray(x), np.asarray(h), np.asarray(W_hidden), np.asarray(b_hidden),
        np.asarray(w_halt), np.asarray(b_halt), np.asarray(W_out), np.asarray(b_out),
    )
    nc = _get_nc()
    res = run_bass_kernel_spmd(nc, in_maps, core_ids=list(range(NCORES)))
    return _gather(res.results)


# revision 39
# speedup vs baseline: 1.2114x; 1.0002x over previous
"""Trainium2 Bass kernel for the batched ACT (adaptive computation time) halting
loop, data-parallel over the batch across 8 NeuronCores.

Math (per batch row b):
    s_0 = h
    s_{t+1} = [xin_t, s_t] @ W_hidden.T + b_hidden     (xin_0 = x+1, else x)
    p_t = sigmoid(s_{t+1} @ w_halt + b_halt)
    halting weights w_t from cumulative p;  sum_t w_t == 1 exactly
    hidden = sum_t w_t s_t;  output = hidden @ W_out.T + b_out  (exact rewrite)
    ponder = (n+1) + R

Rewrites:
  * xproj = x @ Wx.T + b_hidden computed once (Wx = W_hidden[:, :D_IN]);
    per-step matmul contracts only over H (K=2048 not 3072).
  * the t==0 "x+1" flag becomes adding delta = rowsum(Wx) at step 0.
  * output needs only ONE final matmul since sum_t w_t == 1.
  * halt logit from the step INPUT state: s_new.w_halt = s_t.(Ws.T w_halt)
    + (x@Wx.T + b_h).w_halt; u = Ws.T@w_halt on device (bf16), the x-part
    q (+ b_halt) in fp64 on the host -> halt path leaves the copy-dependent
    critical path.
  * state kept transposed sT [H, B_local]; transposes/reshapes host-side.
  * bf16 matmul operands (1 PE cycle/row), fp32 PSUM accumulation.
  * "big tile" layout: state/xproj/hid as single [128, 16*256] SBUF tiles.
  * k-outer matmul order for xproj and step 0 so PE chases weight DMAs.
"""

import numpy as np
import ml_dtypes

B, D_IN, H, D_OUT = 2048, 1024, 2048, 1024
T = 12
EPS = 0.01
NCORES = 8
BL = B // NCORES  # 256 rows per core
P = 128
KT_S = H // P     # 16 k-tiles over H
KT_X = D_IN // P  # 8 k-tiles over D_IN
MT = H // P       # 16 m-tiles over H
NF = 512          # moving free chunk for the final matmul

_BF16 = ml_dtypes.bfloat16


def _to_big(aT, n_free):
    """[K*128, n_free] -> [128, K*n_free], k-tile-major along free dim."""
    k = aT.shape[0] // P
    return np.ascontiguousarray(
        aT.reshape(k, P, n_free).transpose(1, 0, 2).reshape(P, k * n_free)
    )


def _from_big(big, n_free):
    """[128, K*n_free] -> [K*128, n_free]."""
    k = big.shape[1] // n_free
    return np.ascontiguousarray(
        big.reshape(P, k, n_free).transpose(1, 0, 2).reshape(k * P, n_free)
    )


def _build_nc():
    from contextlib import ExitStack

    import concourse.bass as bass
    import concourse.mybir as mybir
    import concourse.tile as tile
    from concourse import bacc

    f32 = mybir.dt.float32
    bf16 = mybir.dt.bfloat16
    ADD = mybir.AluOpType.add
    Sigmoid = mybir.ActivationFunctionType.Sigmoid

    nc = bacc.Bacc("TRN2", target_bir_lowering=False, debug=False)

    xB_d = nc.dram_tensor("xB", [P, KT_X * BL], bf16, kind="ExternalInput")
    hB_d = nc.dram_tensor("hB", [P, KT_S * BL], bf16, kind="ExternalInput")
    WxB_d = nc.dram_tensor("WxB", [P, KT_X * H], bf16, kind="ExternalInput")
    WsB_d = nc.dram_tensor("WsB", [P, KT_S * H], bf16, kind="ExternalInput")
    WoB_d = nc.dram_tensor("WoB", [P, KT_S * D_OUT], bf16, kind="ExternalInput")
    bh_d = nc.dram_tensor("bh", [P, MT], f32, kind="ExternalInput")
    delta_d = nc.dram_tensor("delta", [P, MT], f32, kind="ExternalInput")
    u_d = nc.dram_tensor("u", [P, KT_S], bf16, kind="ExternalInput")
    q_d = nc.dram_tensor("q", [1, BL], f32, kind="ExternalInput")
    q0_d = nc.dram_tensor("q0", [1, BL], f32, kind="ExternalInput")
    bout_d = nc.dram_tensor("bout", [1, D_OUT], bf16, kind="ExternalInput")

    y_d = nc.dram_tensor("y", [BL, D_OUT], f32, kind="ExternalOutput")
    hidB_d = nc.dram_tensor("hidB", [P, KT_S * BL], f32, kind="ExternalOutput")
    pond_d = nc.dram_tensor("ponder", [1, BL], f32, kind="ExternalOutput")

    def bcast_k(ap, k):
        """Broadcast a [128, n] AP to [128, k, n] via a zero-stride dim."""
        return bass.AP(
            tensor=ap.tensor, offset=ap.offset, ap=[ap.ap[0], [0, k], ap.ap[1]]
        )

    with tile.TileContext(nc) as tc, ExitStack() as ctx:
        singles = ctx.enter_context(tc.tile_pool(name="singles", bufs=1))
        wst_pool = ctx.enter_context(tc.tile_pool(name="wst", bufs=1))
        state_pool = ctx.enter_context(tc.tile_pool(name="state", bufs=3))
        xproj_pool = ctx.enter_context(tc.tile_pool(name="xproj", bufs=1))
        hid_pool = ctx.enter_context(tc.tile_pool(name="hid", bufs=1))
        tmp_pool = ctx.enter_context(tc.tile_pool(name="tmp", bufs=1))
        wbc_pool = ctx.enter_context(tc.tile_pool(name="wbc", bufs=2))
        hsmall = ctx.enter_context(tc.tile_pool(name="hsmall", bufs=8))
        cum_pool = ctx.enter_context(tc.tile_pool(name="cum", bufs=2))
        obuf_pool = ctx.enter_context(tc.tile_pool(name="obuf", bufs=4))
        psum_mm = ctx.enter_context(tc.tile_pool(name="psum_mm", bufs=6, space="PSUM"))
        psum_fin = ctx.enter_context(tc.tile_pool(name="psum_fin", bufs=2, space="PSUM"))

        # --- small constants
        bh_t = singles.tile([P, MT], f32)
        nc.sync.dma_start(bh_t[:, :], bh_d[:, :])
        delta_t = singles.tile([P, MT], f32)
        nc.sync.dma_start(delta_t[:, :], delta_d[:, :])
        u_t = singles.tile([P, KT_S], bf16)
        nc.sync.dma_start(u_t[:, :], u_d[:, :])
        q_t = singles.tile([1, BL], f32)
        nc.sync.dma_start(q_t[:, :], q_d[:, :])
        q0_t = singles.tile([1, BL], f32)
        nc.sync.dma_start(q0_t[:, :], q0_d[:, :])
        bout_t = singles.tile([1, D_OUT], bf16)
        nc.sync.dma_start(bout_t[:, :], bout_d[:, :])
        ones_t = singles.tile([1, P], bf16)
        nc.vector.memset(ones_t[:, :], 1.0)

        # warmup: keep PE active during the initial weight DMA so the HAM
        # clock gate reaches 2.4 GHz before xproj/step-0 matmuls start.
        warm_rhs = singles.tile([1, BL], bf16)
        nc.vector.memset(warm_rhs[:, :], 0.0)
        wps = psum_fin.tile([P, BL], f32, tag="fin", name="warm_ps")
        for _ in range(80):
            nc.tensor.matmul(
                wps[:, :], ones_t[:, :], warm_rhs[:, :], start=True, stop=True
            )

        # --- initial state s_0 = h (big layout, bf16)
        sbig = state_pool.tile([P, KT_S * BL], bf16, tag="state")
        nc.sync.dma_start(sbig[:, :], hB_d[:, :])

        # --- xproj = (x @ Wx.T).T + b_hidden, big layout [128, 16*256] f32
        # DMA order: xB + WxB first; k-outer matmuls chase the arriving slices.
        xpbig = xproj_pool.tile([P, KT_S * BL], f32)
        wsb = wst_pool.tile([P, KT_S * H], bf16)

        def mm_group(pairs, n_k, wtile, rtile, kw, out_cb):
            """m-outer pair-tile matmuls (steady state): psum per m-pair.

            The two halves of a pair share a PSUM bank, so their accumulation
            chains must be SEQUENTIAL — start=True clears has_written for the
            whole bank (safe only because half 0 is complete by then).
            """
            for i in pairs:
                ps = psum_mm.tile([P, 2 * BL], f32, tag="ps", name=f"ps_{i}")
                for half in (0, 1):
                    m = 2 * i + half
                    for k in range(n_k):
                        nc.tensor.matmul(
                            ps[:, half * BL : (half + 1) * BL],
                            wtile[:, k * kw + m * P : k * kw + (m + 1) * P],
                            rtile[:, k * BL : (k + 1) * BL],
                            start=(k == 0),
                            stop=(k == n_k - 1),
                        )
                out_cb(i, ps)

        with tc.tile_pool(name="xw", bufs=1) as xw_pool:
            xbt = xw_pool.tile([P, KT_X * BL], bf16, tag="xb")
            nc.sync.dma_start(xbt[:, :], xB_d[:, :])
            wxb = xw_pool.tile([P, KT_X * H], bf16, tag="wxb")
            for k in range(KT_X):
                nc.sync.dma_start(
                    wxb[:, k * H : (k + 1) * H], WxB_d[:, k * H : (k + 1) * H]
                )
            # WsT resident all steps (per-k-slice DMAs, 1MB each)
            for k in range(KT_S):
                nc.sync.dma_start(
                    wsb[:, k * H : (k + 1) * H], WsB_d[:, k * H : (k + 1) * H]
                )

            def xproj_out(i, ps):
                for half in (0, 1):
                    m = 2 * i + half
                    nc.vector.tensor_scalar_add(
                        xpbig[:, m * BL : (m + 1) * BL],
                        ps[:, half * BL : (half + 1) * BL],
                        bh_t[:, m : m + 1],
                    )

            mm_group(range(0, 8), KT_X, wxb, xbt, H, xproj_out)

        # --- W_out.T big tile (created after xw pool closes; reuses its SBUF)
        wout_pool = ctx.enter_context(tc.tile_pool(name="wout", bufs=1))
        wob = wout_pool.tile([P, KT_S * D_OUT], bf16)
        for i in range(4):
            sl = slice(i * (KT_S * D_OUT // 4), (i + 1) * (KT_S * D_OUT // 4))
            nc.sync.dma_start(wob[:, sl], WoB_d[:, sl])

        # --- halting-state accumulators
        pond = singles.tile([1, BL], f32)
        nc.vector.memset(pond[:, :], 0.0)
        cum_prev = cum_pool.tile([1, BL], f32, tag="cum")
        nc.vector.memset(cum_prev[:, :], 0.0)

        hidbig = hid_pool.tile([P, KT_S * BL], f32)
        tmpbig = tmp_pool.tile([P, KT_S * BL], f32)
        pending_hid_add = [False]

        def flush_hid_add():
            # deferred hid += tmp from the PREVIOUS step: emitted after this
            # step's psum evacuations so the (gpsimd-mult-gated) DVE add can't
            # head-of-line block them.
            if pending_hid_add[0]:
                nc.vector.tensor_add(hidbig[:, :], hidbig[:, :], tmpbig[:, :])
                pending_hid_add[0] = False

        # --- the 12-step recurrence
        for t in range(T):
            # halt logit from the INPUT state: p_t = sigmoid(s_t.u + q).
            # two col-groups in separate PSUM banks run concurrently
            # (tile_position col strips), halving the M=1 matmul serial span.
            hpsA = psum_fin.tile([P, BL], f32, tag="fin", name=f"hpsA_{t}")
            hpsB = psum_fin.tile([P, BL], f32, tag="fin", name=f"hpsB_{t}")
            for r in range(KT_S // 2):
                nc.tensor.matmul(
                    hpsA[0:1, :],
                    u_t[:, r : r + 1],
                    sbig[:, r * BL : (r + 1) * BL],
                    start=(r == 0),
                    stop=(r == KT_S // 2 - 1),
                    tile_position=(0, 0),
                )
                k = KT_S // 2 + r
                nc.tensor.matmul(
                    hpsB[32:33, :],
                    u_t[:, k : k + 1],
                    sbig[:, k * BL : (k + 1) * BL],
                    start=(r == 0),
                    stop=(r == KT_S // 2 - 1),
                    tile_position=(0, 32),
                )
            ha = hsmall.tile([1, BL], f32, tag="hs")
            nc.vector.tensor_add(
                ha[:, :], (q0_t if t == 0 else q_t)[:, :], hpsA[0:1, :]
            )
            lg = hsmall.tile([1, BL], f32, tag="hs")
            nc.vector.tensor_add(lg[:, :], ha[:, :], hpsB[32:33, :])
            p_t = hsmall.tile([1, BL], f32, tag="hs")
            nc.scalar.activation(p_t[:, :], lg[:, :], Sigmoid)

            # online halting weights (all [1, BL], partition 0)
            notdone = hsmall.tile([1, BL], f32, tag="hs")
            nc.vector.tensor_scalar(
                notdone[:, :], cum_prev[:, :], 1.0 - EPS, None,
                op0=mybir.AluOpType.is_lt,
            )
            rterm = hsmall.tile([1, BL], f32, tag="hs")
            nc.vector.tensor_scalar(
                rterm[:, :], cum_prev[:, :], -1.0, 1.0,
                op0=mybir.AluOpType.mult, op1=ADD,
            )
            cum_new = cum_pool.tile([1, BL], f32, tag="cum")
            nc.vector.tensor_add(cum_new[:, :], cum_prev[:, :], p_t[:, :])
            col = hsmall.tile([1, BL], f32, tag="hs")
            if t < T - 1:
                nc.vector.tensor_scalar(
                    col[:, :], cum_new[:, :], 1.0 - EPS, None,
                    op0=mybir.AluOpType.is_ge,
                )
            else:
                nc.vector.memset(col[:, :], 1.0)
            # sel = p + col*(rterm - p);  w = sel * notdone
            suba = hsmall.tile([1, BL], f32, tag="hs")
            nc.vector.tensor_sub(suba[:, :], rterm[:, :], p_t[:, :])
            selt = hsmall.tile([1, BL], f32, tag="hs")
            nc.vector.tensor_mul(selt[:, :], col[:, :], suba[:, :])
            sel = hsmall.tile([1, BL], f32, tag="hs")
            nc.vector.tensor_add(sel[:, :], selt[:, :], p_t[:, :])
            w_t = hsmall.tile([1, BL], f32, tag="hs")
            nc.vector.tensor_mul(w_t[:, :], sel[:, :], notdone[:, :])
            isn = hsmall.tile([1, BL], f32, tag="hs")
            nc.vector.tensor_mul(isn[:, :], col[:, :], notdone[:, :])
            nc.vector.tensor_add(pond[:, :], pond[:, :], notdone[:, :])
            tmp2 = hsmall.tile([1, BL], f32, tag="hs")
            nc.vector.tensor_mul(tmp2[:, :], isn[:, :], rterm[:, :])
            nc.vector.tensor_add(pond[:, :], pond[:, :], tmp2[:, :])
            wbc = wbc_pool.tile([P, BL], f32, tag="wbc")
            nc.gpsimd.partition_broadcast(wbc[:, :], w_t[:, :])

            # hidden-state matmuls s_new = Ws @ s_t (+ xproj [+ delta at t=0])
            snew = state_pool.tile([P, KT_S * BL], bf16, tag="state")

            if t == T - 1:
                # flush BEFORE the final step's matmul group: the interleaved
                # quarter mults below write tmpbig and must follow this read
                # of it in DVE queue order (else WAR deadlock).
                flush_hid_add()
                hid_bf = state_pool.tile([P, KT_S * BL], bf16, tag="state")
                KQ = KT_S // 4
                QW = KQ * BL

            def step_out(i, ps, t=t, snew=snew):
                if t == 0:
                    for half in (0, 1):
                        m = 2 * i + half
                        nc.vector.scalar_tensor_tensor(
                            snew[:, m * BL : (m + 1) * BL],
                            ps[:, half * BL : (half + 1) * BL],
                            delta_t[:, m : m + 1],
                            xpbig[:, m * BL : (m + 1) * BL],
                            op0=ADD,
                            op1=ADD,
                        )
                else:
                    nc.vector.tensor_add(
                        snew[:, i * 2 * BL : (i + 1) * 2 * BL],
                        ps[:, :],
                        xpbig[:, i * 2 * BL : (i + 1) * 2 * BL],
                    )
                if t == T - 1 and i % 2 == 1:
                    # quarter q = i//2 covers k-tiles of pairs i-1, i — both
                    # copied now; emit its hid ops (all DVE, no cross-engine
                    # gating) so the final matmul's k-phases start early.
                    q = i // 2
                    sl = slice(q * QW, (q + 1) * QW)
                    sq = snew[:, sl].rearrange("p (k b) -> p k b", k=KQ)
                    tq = tmpbig[:, sl].rearrange("p (k b) -> p k b", k=KQ)
                    nc.vector.tensor_mul(tq, sq, bcast_k(wbc[:, :], KQ))
                    nc.vector.tensor_add(
                        hid_bf[:, sl], hidbig[:, sl], tmpbig[:, sl]
                    )

            mm_group(range(0, 8), KT_S, wsb, sbig, H, step_out)
            flush_hid_add()

            # accumulate hidden: hid += w_t * s_new
            s3 = snew[:, :].rearrange("p (k b) -> p k b", k=KT_S)
            wb3 = bcast_k(wbc[:, :], KT_S)
            if t == 0:
                h3 = hidbig[:, :].rearrange("p (k b) -> p k b", k=KT_S)
                nc.gpsimd.tensor_mul(h3, s3, wb3)
            elif t < T - 1:
                t3 = tmpbig[:, :].rearrange("p (k b) -> p k b", k=KT_S)
                nc.gpsimd.tensor_mul(t3, s3, wb3)
                pending_hid_add[0] = True
            else:
                # quarters were emitted inside step_out (interleaved with the
                # psum copies); only the f32 hidden for the hidB output
                # remains (off the y critical path).
                nc.vector.tensor_add(hidbig[:, :], hidbig[:, :], tmpbig[:, :])

            cum_prev = cum_new
            sbig = snew

        # --- outputs: hidden (big f32), ponder, output = hidden @ W_out.T + b_out
        nc.sync.dma_start(pond_d[:, :], pond[:, :])
        nc.sync.dma_start(hidB_d[:, :], hidbig[:, :])

        # final matmul: 4 psum groups, contraction emitted quarter-phase so PE
        # starts each 4-k chunk as soon as that hid_bf quarter is ready.
        groups = [(mb, nk) for mb in range(BL // P) for nk in range(D_OUT // NF)]
        psos = [
            psum_mm.tile([P, NF], f32, tag="ps", name=f"pso_{g}")
            for g in range(len(groups))
        ]
        for q in range(4):
            for g, (mb, nk) in enumerate(groups):
                for k in range(q * (KT_S // 4), (q + 1) * (KT_S // 4)):
                    nc.tensor.matmul(
                        psos[g][:, :],
                        hid_bf[:, k * BL + mb * P : k * BL + (mb + 1) * P],
                        wob[:, k * D_OUT + nk * NF : k * D_OUT + (nk + 1) * NF],
                        start=(k == 0),
                        stop=False,
                    )
        for g, (mb, nk) in enumerate(groups):
            # += b_out via a K=1 ones-row matmul (keeps the epilogue off DVE)
            nc.tensor.matmul(
                psos[g][:, :],
                ones_t[:, :],
                bout_t[:, nk * NF : (nk + 1) * NF],
                start=False,
                stop=True,
            )
            ob = obuf_pool.tile([P, NF], f32, tag="ob", name=f"ob_{g}")
            nc.vector.tensor_copy(ob[:, :], psos[g][:, :])
            nc.sync.dma_start(
                y_d[mb * P : (mb + 1) * P, nk * NF : (nk + 1) * NF], ob[:, :]
            )

    nc.compile()
    return nc


def _prep_inputs(x, h, W_hidden, b_hidden, w_halt, b_halt, W_out, b_out):
    f32, f64 = np.float32, np.float64
    WT = np.ascontiguousarray(W_hidden.T).astype(_BF16)  # [3072, 2048]
    WxB = _to_big(WT[:D_IN], H)
    WsB = _to_big(WT[D_IN:], H)
    WoB = _to_big(np.ascontiguousarray(W_out.T).astype(_BF16), D_OUT)
    Wx = W_hidden[:, :D_IN]
    delta = Wx.astype(f32).sum(axis=1, dtype=f32)
    bh_t = np.ascontiguousarray(b_hidden.astype(f32).reshape(MT, P).T)
    delta_t = np.ascontiguousarray(delta.reshape(MT, P).T)
    # halt-path host vectors (fp64): u = Ws.T @ w_halt, q = x@(Wx.T@w_halt) + c
    wh64 = w_halt.astype(f64)
    u = (W_hidden[:, D_IN:].T.astype(f64) @ wh64).astype(f32)
    v = Wx.T.astype(f64) @ wh64
    cq = float(b_hidden.astype(f64) @ wh64) + float(b_halt)
    dq = float(delta.astype(f64) @ wh64)
    u_t = np.ascontiguousarray(u.reshape(KT_S, P).T).astype(_BF16)
    bout_b = b_out.astype(_BF16).reshape(1, D_OUT)
    in_maps = []
    for c in range(NCORES):
        xs = x[c * BL : (c + 1) * BL]
        hs = h[c * BL : (c + 1) * BL]
        q = (xs.astype(f64) @ v + cq).astype(f32).reshape(1, BL)
        in_maps.append(
            {
                "xB": _to_big(np.ascontiguousarray(xs.T).astype(_BF16), BL),
                "hB": _to_big(np.ascontiguousarray(hs.T).astype(_BF16), BL),
                "WxB": WxB,
                "WsB": WsB,
                "WoB": WoB,
                "bh": bh_t,
                "delta": delta_t,
                "u": u_t,
                "q": q,
                "q0": (q + np.float32(dq)).astype(f32),
                "bout": bout_b,
            }
        )
    return in_maps


_NC_CACHE = {}


def _get_nc():
    if "nc" not in _NC_CACHE:
        _NC_CACHE["nc"] = _build_nc()
    return _NC_CACHE["nc"]


def _gather(outs):
    y = np.concatenate([np.asarray(outs[c]["y"]) for c in range(NCORES)], axis=0)
    hidden = np.concatenate(
        [_from_big(np.asarray(outs[c]["hidB"]), BL).T for c in range(NCORES)], axis=0
    )
    ponder = np.concatenate(
        [np.asarray(outs[c]["ponder"]).reshape(-1) for c in range(NCORES)], axis=0
    )
    return (
        y.astype(np.float32),
        np.ascontiguousarray(hidden, dtype=np.float32),
        ponder.astype(np.float32),
    )


def kernel(x, h, W_hidden, b_hidden, w_halt, b_halt, W_out, b_out):
    from concourse.bass_utils import run_bass_kernel_spmd

    in_maps = _prep_inputs(
        np.asarray(x), np.asarray(h), np.asarray(W_hidden), np.asarray(b_hidden),
        np.asarray(w_halt), np.asarray(b_halt), np.asarray(W_out), np.asarray(b_out),
    )
    nc = _get_nc()
    res = run_bass_kernel_spmd(nc, in_maps, core_ids=list(range(NCORES)))
    return _gather(res.results)
